# revision 2
# baseline (speedup 1.0000x reference)
"""Causal self-attention (B=2, T=2048, C=1024, H=16) on 8 TRN2 NeuronCores.

Sharding: core c -> batch b = c//4, head-group g = c%4 (4 heads each);
qkv column-sharded per head group, proj column-sharded (Megatron-style)
with y^T AllGathers inside each 4-core batch group.

Key structure (vs the naive version):
  - qkT for heads 0-1 runs contraction-outer so matmuls start while input
    chunks stream in (accumulators borrowed across all three PSUM pools);
    dependency-free zero-matmuls keep the PE clock un-throttled (HAM) in
    DMA-paced gaps.
  - flash-style attention in S^T layout (keys on partitions): scores of a
    head pair run CONCURRENTLY via PE row tiling (K=64 tiles at partitions
    0-63 / 64-127); exp on the scalar engine is the pacing resource.
  - ragged causal exp: diagonal score blocks laid [kc_hi | kc_lo] so the
    causal trim is a contiguous column suffix; AV matmuls skip trimmed
    columns (~10% fewer exp'd elements).
  - denominator via a ones-column in v'; 1/d = exp(-ln d) per (pair,qc) on
    one [1,1024] row; y evacuated early (bf16) so psY stays single-buffered.
  - 8 small per-(pair, query-chunk) AllGathers pipelined behind attention
    (plus a dummy AG first to absorb the startup barrier / first-op setup);
    proj consumes each gather per query chunk, stores stream out per block.
  - windows interleave pair B's first chunk into pair A so v/qkT filler
    matmuls spread evenly; one filler unit per attention j-step.
Matmul inputs bf16 (fp32 PSUM accumulation); rel L2 err ~5.9e-3.
"""

import sys

sys.path.insert(0, "/opt/trn_rl_repo")

import ml_dtypes
import numpy as np

import concourse.bass as bass
import concourse.mybir as mybir
import concourse.tile as tile
from concourse.bass_utils import run_bass_kernel_spmd

B, T, C, H = 2, 2048, 1024, 16
HD = C // H  # 64
HG = 4  # heads per core
CG = HG * HD  # 256
TQ = 512  # query chunk
TK = 128  # key chunk
NQC = T // TQ  # 4
NCC = C // 128  # 8 contraction chunks
SCALE = 1.0 / np.sqrt(HD)

F32 = mybir.dt.float32
BF16 = mybir.dt.bfloat16
BF = ml_dtypes.bfloat16


def _mask_np():
    # block layout [d1 | d0 | d3 | d2], each [128, 512] bf16 with
    # mask_d[k, q] = 1 if k + 128*d <= q (q within the 512-query chunk)
    k = np.arange(TK)[:, None]
    q = np.arange(TQ)[None, :]
    blk = [(k + TK * d <= q).astype(np.float32) for d in range(4)]
    return np.concatenate([blk[1], blk[0], blk[3], blk[2]], axis=1).astype(BF)


_LEGALIZE_SKIP = {
    "InstEventSemaphore",
    "InstCollectiveCompute",
    "InstUnconditionalBranch",
    "InstConditionalBranch",
    "InstRegisterMove",
    "InstCall",
    "InstISA",
}


def _legalize_sync_waits(nc):
    # walrus accepts one semaphore wait per engine instruction; hoist extras
    n = 0
    for bb in nc.main_func.blocks:
        insts = bb.instructions
        k = 0
        while k < len(insts):
            inst = insts[k]
            si = inst.sync_info
            ws = list(si.on_wait) if si and si.on_wait else []
            if type(inst).__name__ not in _LEGALIZE_SKIP and len(ws) > 1:
                for w in ws[:-1]:
                    n += 1
                    ev = mybir.InstEventSemaphore(
                        name=f"xwait_{n}", engine=inst.engine
                    )
                    ev.sync_info = mybir.SyncInfo(on_wait=[w], on_update=[])
                    nc.register_instruction(ev)
                    insts.insert(k, ev)
                    k += 1
                inst.sync_info = mybir.SyncInfo(
                    on_wait=[ws[-1]], on_update=list(si.on_update or [])
                )
            k += 1
    return n


def _build_program():
    nc = bass.Bass()

    xT = nc.declare_dram_parameter("xT", [C, T], BF16, isOutput=False)
    w_qk = nc.declare_dram_parameter("w_qk", [C, 2 * CG], BF16, isOutput=False)
    w_v = nc.declare_dram_parameter("w_v", [C, HG * (HD + 1)], BF16, isOutput=False)
    w_pr = nc.declare_dram_parameter("w_pr", [C, CG], BF16, isOutput=False)
    out = nc.declare_dram_parameter("out", [T, CG], F32, isOutput=True)

    mask_dram = nc.inline_tensor(_mask_np(), name="masks")
    groups = [[0, 1, 2, 3], [4, 5, 6, 7]]

    with tile.TileContext(nc) as tc:
        with (
            tc.tile_pool(name="big", bufs=8) as big_pool,
            tc.tile_pool(name="wqk", bufs=8) as wqk_pool,
            tc.tile_pool(name="wsm", bufs=8) as wsm_pool,
            tc.tile_pool(name="qkT", bufs=4) as qkT_pool,
            tc.tile_pool(name="vp", bufs=16) as vp_pool,
            tc.tile_pool(name="ysb", bufs=2) as y_pool,
            tc.tile_pool(name="yraw", bufs=3) as yraw_pool,
            tc.tile_pool(name="ptile", bufs=6) as p_pool,
            tc.tile_pool(name="yt", bufs=32) as yt_pool,
            tc.tile_pool(name="small", bufs=8) as small_pool,
            tc.tile_pool(name="psS", bufs=2, space="PSUM") as psS_pool,
            tc.tile_pool(name="psY", bufs=1, space="PSUM") as psY_pool,
            tc.tile_pool(name="psA", bufs=2, space="PSUM") as psA_pool,
            tc.tile_pool(name="dram", bufs=1, space="DRAM") as dram_pool,
        ):
            # ---- input loads: xT on scalar queue, weights on sync/gpsimd ----
            xT_sb, w_qk_sb, w_v_sb, w_pr_sb = [], [], [], []
            for cc in range(NCC):
                t_x = big_pool.tile([128, T], BF16, tag="big", name=f"xT{cc}")
                nc.scalar.dma_start(t_x[:], xT[cc * 128 : (cc + 1) * 128, :])
                xT_sb.append(t_x)
                t_w = wqk_pool.tile([128, 2 * CG], BF16, tag="wqk", name=f"wqk{cc}")
                nc.sync.dma_start(t_w[:], w_qk[cc * 128 : (cc + 1) * 128, :])
                w_qk_sb.append(t_w)
                t_v = wsm_pool.tile([128, HG * (HD + 1)], BF16, tag="wv", name=f"wv{cc}")
                nc.gpsimd.dma_start(t_v[:], w_v[cc * 128 : (cc + 1) * 128, :])
                w_v_sb.append(t_v)
            mask_sb = small_pool.tile([128, 4 * TQ], BF16, tag="mask", name="mask_sb")
            nc.sync.dma_start(mask_sb[:], mask_dram[:, :])
            ones64 = small_pool.tile([1, 64], BF16, tag="ones64", name="ones64")
            nc.vector.memset(ones64[:], 1.0)
            # dummy collective: enters the CC queue first and absorbs the
            # startup barrier + first-op setup (~11us) before y is ready
            dum_sb = small_pool.tile([1, 128], BF16, tag="dum", name="dum_sb")
            nc.vector.memset(dum_sb[:], 0.0)
            dum_in = dram_pool.tile([1, 128], BF16, name="dum_in")
            dum_out = dram_pool.tile([4, 128], BF16, name="dum_out")
            nc.sync.dma_start(dum_in[:], dum_sb[:])
            nc.gpsimd.collective_compute(
                "AllGather",
                mybir.AluOpType.bypass,
                replica_groups=groups,
                ins=[dum_in.opt()],
                outs=[dum_out.opt()],
            )
            # w_pr is first needed by proj (~2/3 into the kernel); load it
            # later so it doesn't steal HBM bandwidth from xT/w_qk.
            def emit_wpr_loads():
                for cc in range(NCC):
                    t_p = wsm_pool.tile([128, CG], BF16, tag="wpr", name=f"wpr{cc}")
                    nc.sync.dma_start(t_p[:], w_pr[cc * 128 : (cc + 1) * 128, :])
                    w_pr_sb.append(t_p)
            # PE warm-up: dependency-free zero-matmuls keep the HAM activity
            # window busy (K=8/8) while input chunks stream in, so the real
            # qkT matmuls run at 2.4 GHz as soon as data lands. They add 0
            # into a live qkT accumulator, so no extra PSUM is needed.
            warm_sb = small_pool.tile([128, TQ], BF16, tag="warm", name="warm_sb")
            nc.vector.memset(warm_sb[:], 0.0)

            # ---- qkT tiles [128, T] bf16; mi 0/1: q heads 01/23, 2/3: k ----
            qkT_sb = [None] * 4

            def qT(h):
                return qkT_sb[h // 2][64 * (h % 2) : 64 * (h % 2) + 64, :]

            def kT(h):
                return qkT_sb[2 + h // 2][64 * (h % 2) : 64 * (h % 2) + 64, :]

            # qkT(0,2) contraction-outer: 8 live [128,512] accumulators
            # borrowed as psS bufs (2x[128,1024]) + psY ([128,1024]) + psA.
            def emit_qkT02():
                accS = [
                    psS_pool.tile([128, 2 * TQ], F32, tag="psS", name=f"accS{k}") for k in range(2)
                ]
                accY = psY_pool.tile([128, 2 * TQ], F32, tag="psY", name="accY")
                accA = [
                    psA_pool.tile([128, TQ], F32, tag="psA", name=f"accA{k}") for k in range(2)
                ]

                def acc(mi_i, nj):  # mi_i 0 -> mi0, 1 -> mi2
                    if mi_i == 0:
                        return accS[nj // 2][:, (nj % 2) * TQ : (nj % 2 + 1) * TQ]
                    if nj < 2:
                        return accY[:, nj * TQ : (nj + 1) * TQ]
                    return accA[nj - 2][:]

                def warmup(n):
                    for _ in range(n):
                        nc.tensor.matmul(
                            accS[0][:, 0:TQ],
                            lhsT=warm_sb[:, 0:128],
                            rhs=warm_sb[:],
                            start=False,
                            stop=False,
                            skip_group_check=True,
                        )

                warmup(5)
                for cc in range(NCC):
                    # nj-major so the last chunk finishes nj0 (first query
                    # chunk's q/k) first and attention can start sooner
                    for nj in range(4):
                        for mi_i, mi in enumerate((0, 2)):
                            nc.tensor.matmul(
                                acc(mi_i, nj),
                                lhsT=w_qk_sb[cc][:, mi * 128 : (mi + 1) * 128],
                                rhs=xT_sb[cc][:, nj * TQ : (nj + 1) * TQ],
                                start=(cc == 0),
                                stop=(cc == NCC - 1),
                                skip_group_check=True,
                            )
                    if cc < NCC - 1:
                        warmup(4)
                for mi_i, mi in enumerate((0, 2)):
                    qkT_sb[mi] = qkT_pool.tile(
                        [128, T], BF16, tag="qkT", name=f"qkT{mi}"
                    )
                # nj-outer so the first query chunk's q/k casts come first
                # (the first scores matmul waits on exactly these two)
                for nj in range(4):
                    for mi_i, mi in enumerate((0, 2)):
                        nc.vector.tensor_copy(
                            qkT_sb[mi][:, nj * TQ : (nj + 1) * TQ], acc(mi_i, nj)
                        )

            # qkT(1,3) during attention: one nj block at a time through psA
            def emit_qkT_nj(mi, nj):
                if qkT_sb[mi] is None:
                    qkT_sb[mi] = qkT_pool.tile(
                        [128, T], BF16, tag="qkT", name=f"qkT{mi}"
                    )
                t_qk = qkT_sb[mi]
                ps = psA_pool.tile([128, TQ], F32, tag="psA", name="ps_qk")
                for cc in range(NCC):
                    nc.tensor.matmul(
                        ps[:],
                        lhsT=w_qk_sb[cc][:, mi * 128 : (mi + 1) * 128],
                        rhs=xT_sb[cc][:, nj * TQ : (nj + 1) * TQ],
                        start=(cc == 0),
                        stop=(cc == NCC - 1),
                    )
                nc.vector.tensor_copy(t_qk[:, nj * TQ : (nj + 1) * TQ], ps[:])

            # ---- v' tiles [128, 4*65] bf16, per head [v_h | 1] ----
            vp_sb = [None] * (T // TK)

            def emit_v(ti):
                ps = psA_pool.tile([128, HG * (HD + 1)], F32, tag="psA")
                for cc in range(NCC):
                    nc.tensor.matmul(
                        ps[:],
                        lhsT=xT_sb[cc][:, ti * 128 : (ti + 1) * 128],
                        rhs=w_v_sb[cc][:],
                        start=(cc == 0),
                        stop=(cc == NCC - 1),
                    )
                t_vp = vp_pool.tile(
                    [128, HG * (HD + 1)], BF16, tag="vp", name=f"vp{ti}"
                )
                nc.vector.tensor_copy(t_vp[:], ps[:])
                for h in range(HG):
                    nc.vector.memset(t_vp[:, h * 65 + 64 : h * 65 + 65], 1.0)
                vp_sb[ti] = t_vp

            # ---- attention: pair p = heads (2p, 2p+1), one query chunk ----
            y_sb = [
                y_pool.tile([128, T], BF16, tag="ysb", name=f"ysb{i}") for i in range(2)
            ]

            # zero-matmuls on the PE during scalar-bound stretches keep the
            # HAM activity window busy so real matmuls stay at 2.4 GHz
            # (fragmented PE idle re-throttles the clock to 1.2 GHz)
            def warm_att(n):
                ps_w = psA_pool.tile([128, TQ], F32, tag="psA", name="ps_w")
                for _ in range(n):
                    nc.tensor.matmul(
                        ps_w[:], lhsT=warm_sb[:, 0:128], rhs=warm_sb[:],
                        start=True, stop=True,
                    )

            def emit_att_qc(pair, qc, filler=None, warm=0):
                hA, hB = 2 * pair, 2 * pair + 1
                ps_y = psY_pool.tile([65, 2 * TQ], F32, tag="psY", name="ps_y")
                for j in range(2 * qc + 2):
                    if filler is not None:
                        f = next(filler, None)
                        if f is not None:
                            f()
                        elif warm:
                            warm_att(warm)
                    elif warm:
                        warm_att(warm)
                    kc_lo, kc_hi = 2 * j, 2 * j + 1
                    diag = j - 2 * qc  # >= 0 on the causal diagonal
                    off = 0 if diag < 0 else (128 if diag == 0 else 384)
                    t_hi = 0 if diag < 0 else (128 if diag == 0 else 384)
                    t_lo = 0 if diag < 0 else (0 if diag == 0 else 256)
                    ps_p, p_p = [], []
                    for half, h in enumerate((hA, hB)):
                        ps_s = psS_pool.tile([128, 2 * TQ], F32, tag="psS", name="ps_s")
                        # [hi | lo] so the causal trim is a contiguous suffix
                        nc.tensor.matmul(
                            ps_s[:, 0:TQ],
                            lhsT=kT(h)[:, kc_hi * TK : (kc_hi + 1) * TK],
                            rhs=qT(h)[:, qc * TQ : (qc + 1) * TQ],
                            start=True,
                            stop=True,
                        )
                        nc.tensor.matmul(
                            ps_s[:, TQ : 2 * TQ],
                            lhsT=kT(h)[:, kc_lo * TK : (kc_lo + 1) * TK],
                            rhs=qT(h)[:, qc * TQ : (qc + 1) * TQ],
                            start=True,
                            stop=True,
                        )
                        ps_p.append(ps_s)
                    for half, h in enumerate((hA, hB)):
                        p_t = p_pool.tile([128, 2 * TQ], BF16, tag="ptile", name="p_t")
                        nc.scalar.activation(
                            p_t[:, off:],
                            ps_p[half][:, off:],
                            mybir.ActivationFunctionType.Exp,
                            scale=float(SCALE),
                        )
                        if diag >= 0:
                            nc.vector.tensor_mul(
                                p_t[:, off:],
                                p_t[:, off:],
                                mask_sb[:, diag * 2 * TQ + off : (diag + 1) * 2 * TQ],
                            )
                        p_p.append(p_t)
                    for half, h in enumerate((hA, hB)):
                        p_t = p_p[half]
                        ycol = half * TQ
                        nc.tensor.matmul(
                            ps_y[:, ycol + t_hi : ycol + TQ],
                            lhsT=vp_sb[kc_hi][:, h * 65 : (h + 1) * 65],
                            rhs=p_t[:, t_hi:TQ],
                            start=(j == 0),
                            stop=False,
                            skip_group_check=True,
                        )
                        nc.tensor.matmul(
                            ps_y[:, ycol + t_lo : ycol + TQ],
                            lhsT=vp_sb[kc_lo][:, h * 65 : (h + 1) * 65],
                            rhs=p_t[:, TQ + t_lo : 2 * TQ],
                            start=False,
                            stop=(j == 2 * qc + 1),
                            skip_group_check=True,
                        )
                # evacuate y (undivided, bf16) and the denominator row early
                y_raw = yraw_pool.tile([128, TQ], BF16, tag="yraw", name="y_raw")
                nc.vector.tensor_copy(y_raw[0:64, :], ps_y[0:64, 0:TQ])
                nc.vector.tensor_copy(y_raw[64:128, :], ps_y[0:64, TQ : 2 * TQ])
                den_ln = small_pool.tile([1, 2 * TQ], F32, tag="recipf", bufs=3)
                nc.scalar.activation(
                    den_ln[:], ps_y[64:65, :], mybir.ActivationFunctionType.Ln
                )
                recip = small_pool.tile([1, 2 * TQ], BF16, tag="recip", bufs=3)
                nc.scalar.activation(
                    recip[:],
                    den_ln[:],
                    mybir.ActivationFunctionType.Exp,
                    scale=-1.0,
                )
                # keep PE busy while Ln/recip run on the scalar engine
                warm_att(5)
                # broadcast 1/d to 64 rows per head (col-tiled pair), divide
                ps_b = psA_pool.tile([128, TQ], F32, tag="psA", name="ps_b")
                nc.tensor.matmul(
                    ps_b[0:64, :], lhsT=ones64[:], rhs=recip[:, 0:TQ],
                    start=True, stop=True,
                )
                nc.tensor.matmul(
                    ps_b[64:128, :], lhsT=ones64[:], rhs=recip[:, TQ : 2 * TQ],
                    start=True, stop=True,
                )
                b_sb = small_pool.tile([128, TQ], BF16, tag="bsb", bufs=3)
                nc.vector.tensor_copy(b_sb[:], ps_b[:])
                nc.vector.tensor_mul(
                    y_sb[pair][:, qc * TQ : (qc + 1) * TQ], y_raw[:], b_sb[:]
                )

            # ---- per-(pair,qc) AllGather of y^T slices ----
            # yT_sb[pair][rank][qc]: [128, 512] bf16
            yT_sb = [[[None] * NQC for _ in range(4)] for _ in range(2)]

            def emit_ag(pair, qc):
                i = pair * NQC + qc
                in_cc = dram_pool.tile([128, TQ], BF16, name=f"in_cc{i}")
                out_cc = dram_pool.tile([512, TQ], BF16, name=f"out_cc{i}")
                nc.sync.dma_start(in_cc[:], y_sb[pair][:, qc * TQ : (qc + 1) * TQ])
                nc.gpsimd.collective_compute(
                    "AllGather",
                    mybir.AluOpType.bypass,
                    replica_groups=groups,
                    ins=[in_cc.opt()],
                    outs=[out_cc.opt()],
                )
                for r in range(4):
                    t_y = yt_pool.tile([128, TQ], BF16, tag="yt", name=f"yT{i}_{r}")
                    eng = nc.gpsimd if r % 2 == 0 else nc.sync
                    eng.dma_start(t_y[:], out_cc[r * 128 : (r + 1) * 128, :])
                    yT_sb[pair][r][qc] = t_y

            # ---- proj for one query chunk (all 8 contraction chunks) ----
            # w_pr rows are host-permuted: cc = pair*4 + rank
            def emit_proj(qc):
                for tl in range(TQ // 128):
                    ps = psA_pool.tile([128, CG], F32, tag="psA")
                    for cc in range(NCC):
                        pair, r = cc // 4, cc % 4
                        nc.tensor.matmul(
                            ps[:],
                            lhsT=yT_sb[pair][r][qc][:, tl * 128 : (tl + 1) * 128],
                            rhs=w_pr_sb[cc][:],
                            start=(cc == 0),
                            stop=(cc == NCC - 1),
                        )
                    o_t = small_pool.tile([128, CG], F32, tag="otile", bufs=3)
                    nc.vector.tensor_copy(o_t[:], ps[:])
                    nc.sync.dma_start(
                        out[qc * TQ + tl * 128 : qc * TQ + (tl + 1) * 128, :], o_t[:]
                    )

            # ---- emission schedule ----
            # Fillers: one PE work unit per attention j-step so the scalar
            # engine (exp) never starves at query-chunk boundaries. Pair A
            # has exactly 20 j-steps = 12 v-chunks + 8 qkT nj-blocks.
            # Per-window fillers: one PE work unit per attention j-step so
            # exp never starves and PE load is balanced across windows.
            # v[kc] is emitted before the AV that reads it; qkT nj-block n
            # is emitted before the first pair-B chunk that touches it.
            def F(*units):
                out = []
                for u in units:
                    if u[0] == "v":
                        out.append(lambda ti=u[1]: emit_v(ti))
                    else:
                        out.append(lambda mi=u[1], nj=u[2]: emit_qkT_nj(mi, nj))
                return out

            windows = [
                (0, 0, F(("v", 4), ("v", 5)), 3),
                (0, 1, F(("v", 6), ("v", 7), ("qk", 3, 0), ("qk", 1, 0)), 3),
                (0, 2, F(("v", 8), ("v", 9), ("v", 10), ("v", 11),
                         ("qk", 3, 1), ("qk", 1, 1)), 3),
                (1, 0, F(("qk", 3, 2), ("qk", 1, 2)), 4),
                (0, 3, F(("v", 12), ("v", 13), ("v", 14), ("v", 15),
                         ("qk", 3, 3), ("qk", 1, 3)), 3),
                (1, 1, [], 5),
                (1, 2, [], 5),
                (1, 3, [], 5),
            ]

            emit_qkT02()
            for ti in range(4):
                emit_v(ti)
            for wi, (pair, qc, fl, wa) in enumerate(windows):
                emit_att_qc(pair, qc, filler=iter(fl), warm=wa)
                emit_ag(pair, qc)
                if wi == 0:
                    # w_pr DMAs sit behind the first AG store in the sync
                    # FIFO, so they transfer after the input stream is done
                    emit_wpr_loads()
            for qc in range(NQC):
                emit_proj(qc)

    _legalize_sync_waits(nc)
    return nc


_NC_CACHE = None


def _get_nc():
    global _NC_CACHE
    if _NC_CACHE is None:
        _NC_CACHE = _build_program()
    return _NC_CACHE


def _shard_inputs(x, w_qkv, w_proj):
    """Per-core input maps (bf16). Core c: batch c//4, head group c%4."""
    x = np.asarray(x, np.float32)
    w_qkv = np.asarray(w_qkv, np.float32)
    w_proj = np.asarray(w_proj, np.float32)
    xT = [np.ascontiguousarray(x[b].T).astype(BF) for b in range(B)]  # [C, T]
    wq = w_qkv[:, 0:C]
    wk = w_qkv[:, C : 2 * C]
    wv = w_qkv[:, 2 * C : 3 * C]
    # permuted proj rows: cc = pair*4 + rank -> w_proj[rank*256 + pair*128 :][:128]
    perm = np.concatenate(
        [w_proj[256 * r : 256 * r + 128] for r in range(4)]
        + [w_proj[256 * r + 128 : 256 * (r + 1)] for r in range(4)],
        axis=0,
    )
    in_maps = []
    for c in range(8):
        b, g = c // 4, c % 4
        cols = slice(g * CG, (g + 1) * CG)
        in_maps.append(
            {
                "xT": xT[b],
                "w_qk": np.ascontiguousarray(
                    np.concatenate([wq[:, cols], wk[:, cols]], axis=1)
                ).astype(BF),
                "w_v": np.ascontiguousarray(
                    np.concatenate(
                        [
                            np.concatenate(
                                [
                                    wv[:, g * CG + h * HD : g * CG + (h + 1) * HD],
                                    np.zeros((C, 1), np.float32),
                                ],
                                axis=1,
                            )
                            for h in range(HG)
                        ],
                        axis=1,
                    )
                ).astype(BF),
                "w_pr": np.ascontiguousarray(perm[:, cols]).astype(BF),
            }
        )
    return in_maps


def _assemble(results):
    out = np.empty((B, T, C), np.float32)
    for c in range(8):
        b, g = c // 4, c % 4
        out[b, :, g * CG : (g + 1) * CG] = results[c]["out"]
    return out


def kernel(x, w_qkv, w_proj, **run_kwargs):
    nc = _get_nc()
    in_maps = _shard_inputs(x, w_qkv, w_proj)
    res = run_bass_kernel_spmd(nc, in_maps, core_ids=list(range(8)), **run_kwargs)
    out = _assemble(res.results)
    if run_kwargs:
        return out, res
    return out
